# revision 1
# baseline (speedup 1.0000x reference)
"""Trainium2 Bass kernel for nn_KinematicOperation (kinematic tree forward).

Structure of the (deterministic) problem instance:
  - N = 1 + 2048*768 + 2048*256 atoms.
  - gen0: 2048 chains of 768 atoms rooted at the virtual root (identity HT);
    chain atoms are contiguous: chain c = atoms [1+c*768, 1+(c+1)*768).
  - gen1: 2048 branches of 256 atoms rooted mid-chain (gen0 chain c position
    384); branch atoms contiguous starting at boff = 1 + 2048*768.
  - Local HTs: BOND everywhere except a JUMP at each chain start; root = I.
  - Output: coords[id_idx[a-1]] = prefix_HT(a)[:3, 3] for atoms a = 1..N-1.

Sharding: core k owns gen0 chains [256k, 256(k+1)) and gen1 branches of the
same index range, so the branch-root HT handoff between generations stays
on-core and no collectives are needed.  The final scatter runs on-device via
indirect DMA into a zero-initialized full-size output; the host sums the 8
disjoint per-core outputs.

Device algorithm per generation (all fp32, HTs stored as 3x4 row-major with
implicit [0,0,0,1] bottom row):
  - ACT computes sin/cos (range-wrapped) of the dof angles; DVE assembles the
    local HTs into SBUF.
  - 3-level blocked prefix scan of the HT compose (A,B)->A@B along each chain:
      level1: in-place sequential scan over t within blocks of T atoms,
              lanes = all blocks spread over (partition, free), broadcast-AP
              tensor ops;
      level2: same over u within supers of 8 blocks (lanes = supers);
      level3: sequential exclusive scan over supers (lanes = chains), seeded
              with I (gen0) or the branch-root HT (gen1).
  - Final: translations only: xyz = R_excl_block @ L_local[:, 3] + t_excl,
    written scatter-ready, then indirect-DMA scattered to coords[id_idx].
"""

import os
import sys

import numpy as np

for _p in ("/opt/trn_rl_repo", "/root/.axon_site/_ro/trn_rl_repo"):
    if os.path.isdir(_p) and _p not in sys.path:
        sys.path.insert(0, _p)

# ---------------------------------------------------------------- constants
C0, L0 = 2048, 768
C1, L1 = 2048, 256
N = 1 + C0 * L0 + C1 * L1
BOFF = 1 + C0 * L0
NCORES = 8
P = 128
CHI = 2                      # chains per partition (256 chains per core)
CH0 = C0 // NCORES           # 256 gen0 chains per core
CH1 = C1 // NCORES
A0 = CH0 * L0                # 196608 gen0 atoms per core
A1 = CH1 * L1                # 65536 gen1 atoms per core

# gen0 block geometry: 768 = T*J,  J = S*U supers x blocks
T0, J0, S0, U0 = 12, 64, 8, 8
F0 = CHI * J0                # 128 block-lanes per partition
# gen1 block geometry: 256 = T*J
T1, J1, S1, U1 = 8, 32, 4, 8
F1 = CHI * J1                # 64

PI = float(np.pi)

_CACHE = {}


# ------------------------------------------------------------- device build
def _build_program(repeat=1):
    from concourse import bacc, mybir, tile
    from concourse.bass import AP, IndirectOffsetOnAxis

    f32 = mybir.dt.float32
    i32 = mybir.dt.int32
    MUL = mybir.AluOpType.mult
    SUB = mybir.AluOpType.subtract
    SIN = mybir.ActivationFunctionType.Sin

    nc = bacc.Bacc("TRN2", target_bir_lowering=False, debug=False)

    g0dofs = nc.dram_tensor("g0dofs", [A0, 9], f32, kind="ExternalInput")
    g1dofs = nc.dram_tensor("g1dofs", [A1, 9], f32, kind="ExternalInput")
    jdt_d = nc.dram_tensor("jdt", [P, CHI], i32, kind="ExternalInput")
    kin0_d = nc.dram_tensor("kin0", [P, F0 * T0 * 3], f32, kind="ExternalOutput")
    kin1_d = nc.dram_tensor("kin1", [P, F1 * T1 * 3], f32, kind="ExternalOutput")

    def apx(tl, off, *dims):
        """AP over tile-AP `tl` at free-elem offset `off` with free dims
        [(step, count), ...] (full 128 partitions)."""
        t = tl[:] if not isinstance(tl, AP) else tl
        return AP(t.tensor, t.offset + off, [[t.ap[0][0], P]] + [list(d) for d in dims])

    def compose_1d(vec, lanes, a_off, a_step, b_off, b_step, o_off, o_step,
                   tA, tB, a_tile, b_tile, o_tile):
        """Emit C = A @ B (HT compose) over `lanes` lanes on one free dim.

        a/b/o_(off,step): base free-elem offset and lane stride of the 12-elem
        HTs in their tiles.  tA/tB: two [P, >=lanes*12] temp tiles.
        6 instructions: 3 broadcast mults, 2 adds, 1 translation add.
        """
        for k, dst in ((0, tA), (1, tB)):
            vec.tensor_mul(
                out=apx(dst, 0, (12, lanes), (4, 3), (1, 4)),
                in0=apx(a_tile, a_off + k, (a_step, lanes), (4, 3), (0, 4)),
                in1=apx(b_tile, b_off + 4 * k, (b_step, lanes), (0, 3), (1, 4)),
            )
        vec.tensor_add(
            out=apx(tA, 0, (12, lanes), (1, 12)),
            in0=apx(tA, 0, (12, lanes), (1, 12)),
            in1=apx(tB, 0, (12, lanes), (1, 12)))
        vec.tensor_mul(
            out=apx(tB, 0, (12, lanes), (4, 3), (1, 4)),
            in0=apx(a_tile, a_off + 2, (a_step, lanes), (4, 3), (0, 4)),
            in1=apx(b_tile, b_off + 8, (b_step, lanes), (0, 3), (1, 4)),
        )
        vec.tensor_add(
            out=apx(o_tile, o_off, (o_step, lanes), (1, 12)),
            in0=apx(tA, 0, (12, lanes), (1, 12)),
            in1=apx(tB, 0, (12, lanes), (1, 12)),
        )
        # C[:, i, 3] += A[:, i, 3]
        vec.tensor_add(
            out=apx(o_tile, o_off + 3, (o_step, lanes), (4, 3)),
            in0=apx(o_tile, o_off + 3, (o_step, lanes), (4, 3)),
            in1=apx(a_tile, a_off + 3, (a_step, lanes), (4, 3)),
        )

    def excl_blocks(vec, CS, U, LPS, spx, lp2, rx, tA, tB):
        """rx[cs, u] = spx[cs] @ lp2[cs, u]  (exclusive block prefixes)."""
        for i in range(3):
            for k, dst in ((0, tA), (1, tB)):
                vec.tensor_mul(
                    out=apx(dst, 4 * i, (96, CS), (12, U), (1, 4)),
                    in0=apx(spx, 4 * i + k, (12, CS), (0, U), (0, 4)),
                    in1=apx(lp2, 4 * k, (LPS, CS), (12, U), (1, 4)))
            vec.tensor_add(
                out=apx(tA, 4 * i, (96, CS), (12, U), (1, 4)),
                in0=apx(tA, 4 * i, (96, CS), (12, U), (1, 4)),
                in1=apx(tB, 4 * i, (96, CS), (12, U), (1, 4)))
            vec.tensor_mul(
                out=apx(tB, 4 * i, (96, CS), (12, U), (1, 4)),
                in0=apx(spx, 4 * i + 2, (12, CS), (0, U), (0, 4)),
                in1=apx(lp2, 8, (LPS, CS), (12, U), (1, 4)))
            vec.tensor_add(
                out=apx(rx, 4 * i, (96, CS), (12, U), (1, 4)),
                in0=apx(tA, 4 * i, (96, CS), (12, U), (1, 4)),
                in1=apx(tB, 4 * i, (96, CS), (12, U), (1, 4)))
        vec.tensor_add(
            out=apx(rx, 3, (96, CS), (12, U), (4, 3)),
            in0=apx(rx, 3, (96, CS), (12, U), (4, 3)),
            in1=apx(spx, 3, (12, CS), (0, U), (4, 3)))

    def down_trans(vec, Xt, RXt, OUTt, F, T, t0, t1):
        """xyz[p, f, t, i] = (R_excl[f] @ L[f, t])[i, 3] (translations)."""
        for i in range(3):
            for k, dst in ((0, t0), (1, t1)):
                vec.tensor_mul(
                    out=apx(dst, 0, (T, F), (1, T)),
                    in0=apx(RXt, 4 * i + k, (12, F), (0, T)),
                    in1=apx(Xt, 4 * k + 3, (12, F), (F * 12, T)))
            vec.tensor_add(
                out=apx(t0, 0, (T, F), (1, T)),
                in0=apx(t0, 0, (T, F), (1, T)),
                in1=apx(t1, 0, (T, F), (1, T)))
            vec.tensor_mul(
                out=apx(t1, 0, (T, F), (1, T)),
                in0=apx(RXt, 4 * i + 2, (12, F), (0, T)),
                in1=apx(Xt, 11, (12, F), (F * 12, T)))
            vec.tensor_add(
                out=apx(OUTt, i, (3 * T, F), (3, T)),
                in0=apx(t0, 0, (T, F), (1, T)),
                in1=apx(t1, 0, (T, F), (1, T)))
            vec.tensor_add(
                out=apx(OUTt, i, (3 * T, F), (3, T)),
                in0=apx(OUTt, i, (3 * T, F), (3, T)),
                in1=apx(RXt, 4 * i + 3, (12, F), (0, T)))

    def build_bond(vec, stt, xo, ti, tm1, tm2, tu, tv):
        """Write the 12 bond-HT elements; xo(e)/ti(name)/t*(tile-slice-AP fns)."""
        vec.tensor_scalar_mul(out=xo(0), in0=ti("ct"), scalar1=-1.0)       # -ct
        stt(out=xo(1), in0=ti("st"), scalar=-1.0, in1=ti("cc"),
            op0=MUL, op1=MUL)                                              # -st*cc
        vec.tensor_mul(out=xo(2), in0=ti("st"), in1=ti("sc"))              # st*sc
        vec.tensor_mul(out=xo(3), in0=ti("dd"), in1=xo(0))                 # d*m00
        vec.tensor_mul(out=tm1(), in0=ti("cp"), in1=ti("ct"))              # cp*ct
        vec.tensor_mul(out=tm2(), in0=ti("sp"), in1=ti("ct"))              # sp*ct
        vec.tensor_mul(out=xo(4), in0=ti("cp"), in1=ti("st"))              # cp*st
        vec.tensor_mul(out=tu(), in0=tm1(), in1=ti("cc"))
        vec.tensor_mul(out=tv(), in0=ti("sp"), in1=ti("sc"))
        stt(out=xo(5), in0=tu(), scalar=-1.0, in1=tv(), op0=MUL, op1=SUB)  # -u-v
        vec.tensor_mul(out=tu(), in0=tm1(), in1=ti("sc"))
        vec.tensor_mul(out=tv(), in0=ti("sp"), in1=ti("cc"))
        vec.tensor_sub(out=xo(6), in0=tu(), in1=tv())                      # u-v
        vec.tensor_mul(out=xo(7), in0=ti("dd"), in1=xo(4))                 # d*m10
        vec.tensor_mul(out=xo(8), in0=ti("sp"), in1=ti("st"))              # sp*st
        vec.tensor_mul(out=tu(), in0=tm2(), in1=ti("cc"))
        vec.tensor_mul(out=tv(), in0=ti("cp"), in1=ti("sc"))
        vec.tensor_sub(out=xo(9), in0=tv(), in1=tu())                      # v-u
        vec.tensor_mul(out=tu(), in0=tm2(), in1=ti("sc"))
        vec.tensor_mul(out=tv(), in0=ti("cp"), in1=ti("cc"))
        vec.tensor_add(out=xo(10), in0=tu(), in1=tv())                     # u+v
        vec.tensor_mul(out=xo(11), in0=ti("dd"), in1=xo(8))                # d*m20

    with tile.TileContext(nc) as tc:
      for _rep in range(repeat):
        with tc.tile_pool(name="main", bufs=1) as mp:
            xyz0 = mp.tile([P, F0 * T0 * 3], f32)
            xyz1 = mp.tile([P, F1 * T1 * 3], f32)
            lp2_0 = mp.tile([P, CHI * S0 * (U0 + 1) * 12], f32)
            spx0 = mp.tile([P, CHI * S0 * 12], f32)
            rx0 = mp.tile([P, F0 * 12], f32)
            rbr = mp.tile([P, CHI * 12], f32)
            tA0 = mp.tile([P, F0 * 12], f32)
            tB0 = mp.tile([P, F0 * 12], f32)
            lp2_1 = mp.tile([P, CHI * S1 * (U1 + 1) * 12], f32)
            spx1 = mp.tile([P, CHI * S1 * 12], f32)
            rx1 = mp.tile([P, F1 * 12], f32)
            # jump machinery (tiny)
            jdof = mp.tile([P, CHI * 9], f32)
            jang = mp.tile([P, CHI * 2 * 3], f32)
            jsin = mp.tile([P, CHI * 2 * 3], f32)
            jcos = mp.tile([P, CHI * 2 * 3], f32)
            re_ = mp.tile([P, CHI * 2 * 9], f32)
            rj = mp.tile([P, CHI * 9], f32)
            jht = mp.tile([P, CHI * 12], f32)
            jtmp = mp.tile([P, CHI * 2 * 9], f32)
            jdt = mp.tile([P, CHI], i32)
            jmask = mp.tile([P, CHI], f32)

            nc.sync.dma_start(out=jdt[:], in_=jdt_d[:])

            V = nc.vector
            stt = V.scalar_tensor_tensor

            # ======================= GEN 0 =======================
            with tc.tile_pool(name="px0", bufs=1) as px:
                X0 = px.tile([P, T0 * F0 * 12], f32)

                for chi in range(CHI):
                    with tc.tile_pool(name=f"pfront{chi}", bufs=1) as fp:
                        dof_c = fp.tile([P, L0 * 9], f32, name=f"dof_c{chi}")
                        trig = {nm: fp.tile([P, L0], f32, name=f"trg{chi}_{nm}")
                                for nm in ("cp", "sp", "ct", "st", "cc", "sc",
                                           "dd")}
                        wv = fp.tile([P, L0], f32, name=f"wv{chi}")
                        tm1 = fp.tile([P, L0], f32, name=f"tm1_{chi}")
                        tm2 = fp.tile([P, L0], f32, name=f"tm2_{chi}")
                        tu = fp.tile([P, L0], f32, name=f"tu{chi}")
                        tv = fp.tile([P, L0], f32, name=f"tv{chi}")

                        src = AP(g0dofs, chi * P * L0 * 9,
                                 [[L0 * 9, P], [1, L0 * 9]])
                        nc.sync.dma_start(out=dof_c[:], in_=src)

                        def dcol(col):
                            return apx(dof_c, col, (9, L0))

                        for col, cosn, sinn in ((0, "cp", "sp"), (1, "ct", "st"),
                                                (3, "cc", "sc")):
                            for shift, nm in ((0.0, sinn), (PI / 2, cosn)):
                                V.add_range_wrap(out=wv[:], in_=dcol(col),
                                                 shift=shift, bound=PI,
                                                 period=2 * PI)
                                nc.scalar.activation(out=trig[nm][:], in_=wv[:],
                                                     func=SIN)
                        nc.scalar.copy(out=trig["dd"][:], in_=dcol(2))
                        V.tensor_copy(out=jdof[:, chi * 9:(chi + 1) * 9],
                                      in_=dof_c[:, 0:9])

                        xbase = chi * J0 * 12

                        def xo(e, _b=xbase):
                            return apx(X0, _b + e, (12, J0), (F0 * 12, T0))

                        def ti(nm):
                            return apx(trig[nm], 0, (T0, J0), (1, T0))

                        def mk(tl):
                            return lambda: apx(tl, 0, (T0, J0), (1, T0))

                        build_bond(V, stt, xo, ti, mk(tm1), mk(tm2), mk(tu),
                                   mk(tv))

                # ---- JUMP HTs for chain-start lanes ----
                V.tensor_copy(out=jang[:], in_=apx(jdof, 3, (9, CHI), (3, 2),
                                                   (1, 3)))
                V.add_range_wrap(out=jsin[:], in_=jang[:], shift=0.0, bound=PI,
                                 period=2 * PI)
                nc.scalar.activation(out=jsin[:], in_=jsin[:], func=SIN)
                V.add_range_wrap(out=jcos[:], in_=jang[:], shift=PI / 2,
                                 bound=PI, period=2 * PI)
                nc.scalar.activation(out=jcos[:], in_=jcos[:], func=SIN)

                CR = CHI * 2

                def sc_(tl, ang):
                    return apx(tl, ang, (3, CR))

                def re(e):
                    return apx(re_, e, (9, CR))

                def jt1(e):
                    return apx(jtmp, e, (9, CR))

                sa = lambda: sc_(jsin, 0)
                sb = lambda: sc_(jsin, 1)
                s_c = lambda: sc_(jsin, 2)
                ca = lambda: sc_(jcos, 0)
                cb = lambda: sc_(jcos, 1)
                c_c = lambda: sc_(jcos, 2)
                # R = Rz(c)Ry(b)Rx(a) per (chi, rot) lane
                V.tensor_mul(out=re(0), in0=c_c(), in1=cb())
                V.tensor_mul(out=jt1(0), in0=sb(), in1=sa())       # sb*sa
                V.tensor_mul(out=jt1(1), in0=sb(), in1=ca())       # sb*ca
                V.tensor_mul(out=jt1(2), in0=c_c(), in1=jt1(0))
                V.tensor_mul(out=jt1(3), in0=s_c(), in1=ca())
                V.tensor_sub(out=re(1), in0=jt1(2), in1=jt1(3))
                V.tensor_mul(out=jt1(2), in0=c_c(), in1=jt1(1))
                V.tensor_mul(out=jt1(3), in0=s_c(), in1=sa())
                V.tensor_add(out=re(2), in0=jt1(2), in1=jt1(3))
                V.tensor_mul(out=re(3), in0=s_c(), in1=cb())
                V.tensor_mul(out=jt1(2), in0=s_c(), in1=jt1(0))
                V.tensor_mul(out=jt1(3), in0=c_c(), in1=ca())
                V.tensor_add(out=re(4), in0=jt1(2), in1=jt1(3))
                V.tensor_mul(out=jt1(2), in0=s_c(), in1=jt1(1))
                V.tensor_mul(out=jt1(3), in0=c_c(), in1=sa())
                V.tensor_sub(out=re(5), in0=jt1(2), in1=jt1(3))
                V.tensor_scalar_mul(out=re(6), in0=sb(), scalar1=-1.0)
                V.tensor_mul(out=re(7), in0=cb(), in1=sa())
                V.tensor_mul(out=re(8), in0=cb(), in1=ca())
                # rj = R1 @ R2 (3x3), lanes = chi
                V.tensor_mul(
                    out=apx(rj, 0, (9, CHI), (3, 3), (1, 3)),
                    in0=apx(re_, 0, (18, CHI), (3, 3), (0, 3)),
                    in1=apx(re_, 9, (18, CHI), (0, 3), (1, 3)))
                V.tensor_mul(
                    out=apx(jtmp, 0, (9, CHI), (3, 3), (1, 3)),
                    in0=apx(re_, 1, (18, CHI), (3, 3), (0, 3)),
                    in1=apx(re_, 12, (18, CHI), (0, 3), (1, 3)))
                V.tensor_add(out=rj[:, : CHI * 9], in0=rj[:, : CHI * 9],
                             in1=jtmp[:, : CHI * 9])
                V.tensor_mul(
                    out=apx(jtmp, 0, (9, CHI), (3, 3), (1, 3)),
                    in0=apx(re_, 2, (18, CHI), (3, 3), (0, 3)),
                    in1=apx(re_, 15, (18, CHI), (0, 3), (1, 3)))
                V.tensor_add(out=rj[:, : CHI * 9], in0=rj[:, : CHI * 9],
                             in1=jtmp[:, : CHI * 9])
                V.tensor_copy(out=apx(jht, 0, (12, CHI), (4, 3), (1, 3)),
                              in_=apx(rj, 0, (9, CHI), (3, 3), (1, 3)))
                V.tensor_copy(out=apx(jht, 3, (12, CHI), (4, 3)),
                              in_=apx(jdof, 0, (9, CHI), (1, 3)))
                # blend: X[start] += mask * (jump - X[start]),  mask = (jdt==1)
                V.tensor_scalar(out=jmask[:], in0=jdt[:], scalar1=1,
                                scalar2=None, op0=mybir.AluOpType.is_equal)
                V.tensor_sub(out=apx(jtmp, 0, (12, CHI), (1, 12)),
                             in0=apx(jht, 0, (12, CHI), (1, 12)),
                             in1=apx(X0, 0, (J0 * 12, CHI), (1, 12)))
                V.tensor_mul(out=apx(jtmp, 0, (12, CHI), (1, 12)),
                             in0=apx(jtmp, 0, (12, CHI), (1, 12)),
                             in1=apx(jmask, 0, (1, CHI), (0, 12)))
                V.tensor_add(out=apx(X0, 0, (J0 * 12, CHI), (1, 12)),
                             in0=apx(X0, 0, (J0 * 12, CHI), (1, 12)),
                             in1=apx(jtmp, 0, (12, CHI), (1, 12)))

                # ---- level-1 bottom scan (in place over X0 slabs) ----
                for t in range(1, T0):
                    compose_1d(V, F0,
                               a_off=(t - 1) * F0 * 12, a_step=12,
                               b_off=t * F0 * 12, b_step=12,
                               o_off=t * F0 * 12, o_step=12,
                               tA=tA0, tB=tB0,
                               a_tile=X0, b_tile=X0, o_tile=X0)

                # ---- level-2: supers of 8 blocks; lp2[cs, 0] = I ----
                LPS = (U0 + 1) * 12
                BPO = (T0 - 1) * F0 * 12
                V.memset(lp2_0[:], 0.0)
                V.memset(apx(lp2_0, 0, (LPS, CHI * S0), (5, 3)), 1.0)
                nc.scalar.copy(out=apx(lp2_0, 12, (LPS, CHI * S0), (1, 12)),
                               in_=apx(X0, BPO, (U0 * 12, CHI * S0), (1, 12)))
                for u in range(1, U0):
                    compose_1d(V, CHI * S0,
                               a_off=u * 12, a_step=LPS,
                               b_off=BPO + u * 12, b_step=U0 * 12,
                               o_off=(u + 1) * 12, o_step=LPS,
                               tA=tA0, tB=tB0,
                               a_tile=lp2_0, b_tile=X0, o_tile=lp2_0)

                # ---- level-3: exclusive scan over supers, seeded with I ----
                V.memset(spx0[:], 0.0)
                V.memset(apx(spx0, 0, (S0 * 12, CHI), (5, 3)), 1.0)
                for s in range(1, S0):
                    compose_1d(V, CHI,
                               a_off=(s - 1) * 12, a_step=S0 * 12,
                               b_off=(s - 1) * LPS + U0 * 12, b_step=S0 * LPS,
                               o_off=s * 12, o_step=S0 * 12,
                               tA=tA0, tB=tB0,
                               a_tile=spx0, b_tile=lp2_0, o_tile=spx0)

                excl_blocks(V, CHI * S0, U0, LPS, spx0, lp2_0, rx0, tA0, tB0)

                # branch roots: rbr = rx0[block 32] @ X0[t=0, j=32]
                compose_1d(V, CHI,
                           a_off=32 * 12, a_step=J0 * 12,
                           b_off=32 * 12, b_step=J0 * 12,
                           o_off=0, o_step=12,
                           tA=tA0, tB=tB0,
                           a_tile=rx0, b_tile=X0, o_tile=rbr)

                down_trans(V, X0, rx0, xyz0, F0, T0, tA0, tB0)

            # ---- write gen0 kin coords (host applies the id_idx permutation)
            nc.sync.dma_start(out=kin0_d[:], in_=xyz0[:])

            # ======================= GEN 1 =======================
            with tc.tile_pool(name="pfront1", bufs=1) as fp1:
                dof1 = fp1.tile([P, CHI * L1 * 9], f32)
                trig1 = {nm: fp1.tile([P, CHI * L1], f32, name=f"trig1_{nm}")
                         for nm in ("cp", "sp", "ct", "st", "cc", "sc", "dd")}
                X1 = fp1.tile([P, T1 * F1 * 12], f32)
                w1 = fp1.tile([P, CHI * L1], f32)
                tm1b = fp1.tile([P, CHI * L1], f32)
                tm2b = fp1.tile([P, CHI * L1], f32)
                tub = fp1.tile([P, CHI * L1], f32)
                tvb = fp1.tile([P, CHI * L1], f32)

                src = AP(g1dofs, 0, [[L1 * 9, P], [P * L1 * 9, CHI], [1, L1 * 9]])
                dst = AP(dof1[:].tensor, dof1[:].offset,
                         [[dof1[:].ap[0][0], P], [L1 * 9, CHI], [1, L1 * 9]])
                nc.sync.dma_start(out=dst, in_=src)

                def dcol1(col):
                    return AP(dof1[:].tensor, dof1[:].offset + col,
                              [[dof1[:].ap[0][0], P], [L1 * 9, CHI], [9, L1]])

                for col, cosn, sinn in ((0, "cp", "sp"), (1, "ct", "st"),
                                        (3, "cc", "sc")):
                    for shift, nm in ((0.0, sinn), (PI / 2, cosn)):
                        V.add_range_wrap(out=w1[:], in_=dcol1(col),
                                         shift=shift, bound=PI, period=2 * PI)
                        nc.scalar.activation(out=trig1[nm][:], in_=w1[:],
                                             func=SIN)
                nc.scalar.copy(out=trig1["dd"][:], in_=dcol1(2))

                def xo1(e):
                    return apx(X1, e, (J1 * 12, CHI), (12, J1), (F1 * 12, T1))

                def ti1(nm):
                    return apx(trig1[nm], 0, (L1, CHI), (T1, J1), (1, T1))

                def mk1(tl):
                    return lambda: apx(tl, 0, (L1, CHI), (T1, J1), (1, T1))

                build_bond(V, stt, xo1, ti1, mk1(tm1b), mk1(tm2b), mk1(tub),
                           mk1(tvb))

                for t in range(1, T1):
                    compose_1d(V, F1,
                               a_off=(t - 1) * F1 * 12, a_step=12,
                               b_off=t * F1 * 12, b_step=12,
                               o_off=t * F1 * 12, o_step=12,
                               tA=tA0, tB=tB0,
                               a_tile=X1, b_tile=X1, o_tile=X1)

                LPS1 = (U1 + 1) * 12
                BPO1 = (T1 - 1) * F1 * 12
                V.memset(lp2_1[:], 0.0)
                V.memset(apx(lp2_1, 0, (LPS1, CHI * S1), (5, 3)), 1.0)
                nc.scalar.copy(out=apx(lp2_1, 12, (LPS1, CHI * S1), (1, 12)),
                               in_=apx(X1, BPO1, (U1 * 12, CHI * S1), (1, 12)))
                for u in range(1, U1):
                    compose_1d(V, CHI * S1,
                               a_off=u * 12, a_step=LPS1,
                               b_off=BPO1 + u * 12, b_step=U1 * 12,
                               o_off=(u + 1) * 12, o_step=LPS1,
                               tA=tA0, tB=tB0,
                               a_tile=lp2_1, b_tile=X1, o_tile=lp2_1)

                # level-3 gen1: seeded with branch roots
                V.tensor_copy(out=apx(spx1, 0, (S1 * 12, CHI), (1, 12)),
                              in_=apx(rbr, 0, (12, CHI), (1, 12)))
                for s in range(1, S1):
                    compose_1d(V, CHI,
                               a_off=(s - 1) * 12, a_step=S1 * 12,
                               b_off=(s - 1) * LPS1 + U1 * 12, b_step=S1 * LPS1,
                               o_off=s * 12, o_step=S1 * 12,
                               tA=tA0, tB=tB0,
                               a_tile=spx1, b_tile=lp2_1, o_tile=spx1)

                excl_blocks(V, CHI * S1, U1, LPS1, spx1, lp2_1, rx1, tA0, tB0)

                down_trans(V, X1, rx1, xyz1, F1, T1, tA0, tB0)

            nc.sync.dma_start(out=kin1_d[:], in_=xyz1[:])

    nc.compile()
    return nc


def get_program(repeat=1):
    key = ("nc", repeat)
    if key not in _CACHE:
        _CACHE[key] = _build_program(repeat)
    return _CACHE[key]


# ------------------------------------------------------------------- host
def _shard_inputs(dofs, doftype):
    """Build the 8 per-core input maps (lane order (p, chi, j, t))."""
    in_maps = []
    chain_starts = 1 + np.arange(C0, dtype=np.int64) * L0
    jdt_all = np.ascontiguousarray(doftype[chain_starts])
    for core in range(NCORES):
        g0 = dofs[1 + core * A0: 1 + (core + 1) * A0]
        g1 = dofs[BOFF + core * A1: BOFF + (core + 1) * A1]
        jdt = np.ascontiguousarray(
            jdt_all[core * CH0:(core + 1) * CH0].reshape(CHI, P).T)
        in_maps.append({
            "g0dofs": np.ascontiguousarray(g0),
            "g1dofs": np.ascontiguousarray(g1),
            "jdt": jdt,
        })
    return in_maps


def _lane_ids(id_idx, core):
    """id_idx values of this core's atoms in device lane order (p, f, t)."""
    ids0 = (id_idx[core * A0:(core + 1) * A0]
            .reshape(CHI, P, L0).transpose(1, 0, 2).ravel())
    ids1 = (id_idx[BOFF - 1 + core * A1: BOFF - 1 + (core + 1) * A1]
            .reshape(CHI, P, L1).transpose(1, 0, 2).ravel())
    return ids0, ids1


def _structure_ok(doftype, gen0_paths, gen1_paths):
    chain_starts = 1 + np.arange(C0, dtype=np.int64) * L0
    g0 = np.concatenate(
        [np.zeros((C0, 1), np.int64), chain_starts[:, None] + np.arange(L0)],
        axis=1)
    if not np.array_equal(gen0_paths, g0.astype(gen0_paths.dtype)):
        return False
    branch_roots = chain_starts + L0 // 2
    g1 = np.concatenate(
        [branch_roots[:, None],
         BOFF + (np.arange(C1, dtype=np.int64) * L1)[:, None] + np.arange(L1)],
        axis=1)
    if not np.array_equal(gen1_paths, g1.astype(gen1_paths.dtype)):
        return False
    if doftype[0] != 0:
        return False
    dt = doftype.copy()
    dt[chain_starts] = 2
    if not np.all(dt[1:] == 2):
        return False
    return True


def _numpy_fallback(dofs, doftype, gen0_paths, gen1_paths, id_idx):
    """Exact numpy port of the reference (slow path, safety net)."""
    def rx(a):
        c, s = np.cos(a), np.sin(a)
        o, z = np.ones_like(a), np.zeros_like(a)
        return np.stack([np.stack([o, z, z, z], -1), np.stack([z, c, -s, z], -1),
                         np.stack([z, s, c, z], -1), np.stack([z, z, z, o], -1)], -2)

    def ry(a):
        c, s = np.cos(a), np.sin(a)
        o, z = np.ones_like(a), np.zeros_like(a)
        return np.stack([np.stack([c, z, s, z], -1), np.stack([z, o, z, z], -1),
                         np.stack([-s, z, c, z], -1), np.stack([z, z, z, o], -1)], -2)

    def rz(a):
        c, s = np.cos(a), np.sin(a)
        o, z = np.ones_like(a), np.zeros_like(a)
        return np.stack([np.stack([c, -s, z, z], -1), np.stack([s, c, z, z], -1),
                         np.stack([z, z, o, z], -1), np.stack([z, z, z, o], -1)], -2)

    def trans(x, y, z):
        o, zr = np.ones_like(x), np.zeros_like(x)
        return np.stack([np.stack([o, zr, zr, x], -1), np.stack([zr, o, zr, y], -1),
                         np.stack([zr, zr, o, z], -1), np.stack([zr, zr, zr, o], -1)], -2)

    dofs = dofs.astype(np.float32)
    phi_p, theta, d, phi_c = dofs[:, 0], dofs[:, 1], dofs[:, 2], dofs[:, 3]
    z = np.zeros_like(d)
    bond = rx(phi_p) @ rz(np.pi - theta) @ trans(d, z, z) @ rx(phi_c)
    rot = lambda a, b, c: rz(c) @ ry(b) @ rx(a)
    jump = (trans(dofs[:, 0], dofs[:, 1], dofs[:, 2])
            @ rot(dofs[:, 3], dofs[:, 4], dofs[:, 5])
            @ rot(dofs[:, 6], dofs[:, 7], dofs[:, 8]))
    eye = np.broadcast_to(np.eye(4, dtype=dofs.dtype), bond.shape)
    dt = doftype[:, None, None]
    hts = np.where(dt == 1, jump, np.where(dt == 2, bond, eye)).astype(np.float32)
    for paths in (gen0_paths, gen1_paths):
        seg = hts[paths]
        out = np.empty_like(seg)
        out[:, 0] = seg[:, 0]
        for i in range(1, seg.shape[1]):
            out[:, i] = out[:, i - 1] @ seg[:, i]
        hts[paths] = out
    kincoords = hts[:, :3, 3]
    coords = np.zeros((N - 1, 3), dtype=dofs.dtype)
    coords[np.asarray(id_idx)] = kincoords[1:]
    return coords


def kernel(dofs, doftype, gen0_paths, gen1_paths, id_idx):
    dofs = np.asarray(dofs, dtype=np.float32)
    doftype = np.asarray(doftype, dtype=np.int32)
    gen0_paths = np.asarray(gen0_paths)
    gen1_paths = np.asarray(gen1_paths)
    id_idx = np.asarray(id_idx, dtype=np.int32)

    if not _structure_ok(doftype, gen0_paths, gen1_paths):
        return _numpy_fallback(dofs, doftype, gen0_paths, gen1_paths, id_idx)

    from concourse.bass_utils import run_bass_kernel_spmd

    nc = get_program()
    in_maps = _shard_inputs(dofs, doftype)
    res = run_bass_kernel_spmd(nc, in_maps, core_ids=list(range(NCORES)))
    out = np.empty((N - 1, 3), dtype=np.float32)
    for core in range(NCORES):
        ids0, ids1 = _lane_ids(id_idx, core)
        out[ids0] = res.results[core]["kin0"].reshape(-1, 3)
        out[ids1] = res.results[core]["kin1"].reshape(-1, 3)
    return out



# revision 4
# speedup vs baseline: 1.2468x; 1.2468x over previous
"""Trainium2 Bass kernel for nn_KinematicOperation (kinematic tree forward).

Structure of the (deterministic) problem instance:
  - N = 1 + 2048*768 + 2048*256 atoms.
  - gen0: 2048 chains of 768 atoms rooted at the virtual root (identity HT);
    chain atoms are contiguous: chain c = atoms [1+c*768, 1+(c+1)*768).
  - gen1: 2048 branches of 256 atoms rooted mid-chain (gen0 chain c position
    384); branch atoms contiguous starting at boff = 1 + 2048*768.
  - Local HTs: BOND everywhere except a JUMP at each chain start; root = I.
  - Output: coords[id_idx[a-1]] = prefix_HT(a)[:3, 3] for atoms a = 1..N-1.

Sharding: core k owns gen0 chains [256k, 256(k+1)) and gen1 branches of the
same index range, so the branch-root HT handoff between generations stays
on-core and no collectives are needed.  Host pre-slices bond dof columns
(0..3 of 9) and gathers jump rows, shrinking input DMA.

Device algorithm per generation (fp32; rotations stored as 3x3 row-major,
translations separately):
  - ACT computes sin/cos (one DVE range-wrap per angle; cos as
    sin(pi/2 - |w|)); DVE assembles the local 3x3 rotations into SBUF.
  - 3-level blocked prefix scan along each chain:
      level1: rotation-only scan propagating ROWS 0,1 (6 elems) in place;
      translations via the NeRF identity local_t = d * col0(localR):
      t_glob(p) = sum_{q<=p} d_q * col0(R_glob_q), so in-block translations
      are prefix SUMS of w = d * col0(R_inblock) (col0 z-comp from a cross
      product), then level2/3 compose full 3x4 block HTs (tiny), and the
      final transform applies block-exclusive R,t to the in-block cumsums.
  - Output xyz written scatter-ready; host applies the id_idx permutation.
"""

import os
import sys

import numpy as np

for _p in ("/opt/trn_rl_repo", "/root/.axon_site/_ro/trn_rl_repo"):
    if os.path.isdir(_p) and _p not in sys.path:
        sys.path.insert(0, _p)

# ---------------------------------------------------------------- constants
C0, L0 = 2048, 768
C1, L1 = 2048, 256
N = 1 + C0 * L0 + C1 * L1
BOFF = 1 + C0 * L0
NCORES = 8
P = 128
CHI = 2                      # chains per partition (256 chains per core)
CH0 = C0 // NCORES
CH1 = C1 // NCORES
A0 = CH0 * L0                # 196608 gen0 atoms per core
A1 = CH1 * L1                # 65536 gen1 atoms per core

# block geometry: L = T*J,  J = S*U supers x blocks
T0, J0, S0, U0 = 12, 64, 8, 8
F0 = CHI * J0                # 128 block-lanes per partition
T1, J1, S1, U1 = 8, 32, 4, 8
F1 = CHI * J1                # 64

PI = float(np.pi)

_CACHE = {}


# ------------------------------------------------------------- device build
def _build_program(repeat=1):
    from concourse import bacc, mybir, tile
    from concourse.bass import AP

    f32 = mybir.dt.float32
    MUL = mybir.AluOpType.mult
    SUB = mybir.AluOpType.subtract
    SIN = mybir.ActivationFunctionType.Sin
    ABS = mybir.ActivationFunctionType.Abs

    nc = bacc.Bacc("TRN2", target_bir_lowering=False, debug=False)

    b0_d = nc.dram_tensor("b0", [A0, 4], f32, kind="ExternalInput")
    b1_d = nc.dram_tensor("b1", [A1, 4], f32, kind="ExternalInput")
    jd_d = nc.dram_tensor("jd", [P, CHI * 9], f32, kind="ExternalInput")
    kin0_d = nc.dram_tensor("kin0", [P, F0 * T0 * 3], f32, kind="ExternalOutput")
    kin1_d = nc.dram_tensor("kin1", [P, F1 * T1 * 3], f32, kind="ExternalOutput")

    def apx(tl, off, *dims):
        """AP over tile-AP `tl` at free-elem offset `off` with free dims
        [(step, count), ...] (full 128 partitions)."""
        t = tl[:] if not isinstance(tl, AP) else tl
        return AP(t.tensor, t.offset + off, [[t.ap[0][0], P]] + [list(d) for d in dims])

    def compose_1d(vec, lanes, a_off, a_step, b_off, b_step, o_off, o_step,
                   tA, tB, a_tile, b_tile, o_tile):
        """C = A @ B (3x4 HT compose, 12-elem row-major layout) over lanes."""
        for k, dst in ((0, tA), (1, tB)):
            vec.tensor_mul(
                out=apx(dst, 0, (12, lanes), (4, 3), (1, 4)),
                in0=apx(a_tile, a_off + k, (a_step, lanes), (4, 3), (0, 4)),
                in1=apx(b_tile, b_off + 4 * k, (b_step, lanes), (0, 3), (1, 4)),
            )
        vec.tensor_add(
            out=apx(tA, 0, (12, lanes), (1, 12)),
            in0=apx(tA, 0, (12, lanes), (1, 12)),
            in1=apx(tB, 0, (12, lanes), (1, 12)))
        vec.tensor_mul(
            out=apx(tB, 0, (12, lanes), (4, 3), (1, 4)),
            in0=apx(a_tile, a_off + 2, (a_step, lanes), (4, 3), (0, 4)),
            in1=apx(b_tile, b_off + 8, (b_step, lanes), (0, 3), (1, 4)),
        )
        vec.tensor_add(
            out=apx(o_tile, o_off, (o_step, lanes), (1, 12)),
            in0=apx(tA, 0, (12, lanes), (1, 12)),
            in1=apx(tB, 0, (12, lanes), (1, 12)),
        )
        vec.tensor_add(
            out=apx(o_tile, o_off + 3, (o_step, lanes), (4, 3)),
            in0=apx(o_tile, o_off + 3, (o_step, lanes), (4, 3)),
            in1=apx(a_tile, a_off + 3, (a_step, lanes), (4, 3)),
        )

    def excl_blocks(vec, CS, U, LPS, spx, lp2, rx, tA, tB):
        """rx[cs, u] = spx[cs] @ lp2[cs, u]  (exclusive block prefixes)."""
        for i in range(3):
            for k, dst in ((0, tA), (1, tB)):
                vec.tensor_mul(
                    out=apx(dst, 4 * i, (96, CS), (12, U), (1, 4)),
                    in0=apx(spx, 4 * i + k, (12, CS), (0, U), (0, 4)),
                    in1=apx(lp2, 4 * k, (LPS, CS), (12, U), (1, 4)))
            vec.tensor_add(
                out=apx(tA, 4 * i, (96, CS), (12, U), (1, 4)),
                in0=apx(tA, 4 * i, (96, CS), (12, U), (1, 4)),
                in1=apx(tB, 4 * i, (96, CS), (12, U), (1, 4)))
            vec.tensor_mul(
                out=apx(tB, 4 * i, (96, CS), (12, U), (1, 4)),
                in0=apx(spx, 4 * i + 2, (12, CS), (0, U), (0, 4)),
                in1=apx(lp2, 8, (LPS, CS), (12, U), (1, 4)))
            vec.tensor_add(
                out=apx(rx, 4 * i, (96, CS), (12, U), (1, 4)),
                in0=apx(tA, 4 * i, (96, CS), (12, U), (1, 4)),
                in1=apx(tB, 4 * i, (96, CS), (12, U), (1, 4)))
        vec.tensor_add(
            out=apx(rx, 3, (96, CS), (12, U), (4, 3)),
            in0=apx(rx, 3, (96, CS), (12, U), (4, 3)),
            in1=apx(spx, 3, (12, CS), (0, U), (4, 3)))

    # ---- generation emitters (engine-parameterized) ----

    def emit_trig(V, S, dof, trig, aw, L, halfpi):
        """Wrap + sin/cos planes for angle cols 0,1,3 of the 4-wide dofs.
        One DVE range-wrap per angle (into the cos plane as scratch), sin on
        ACT, cos = sin(pi/2 - |w|) on ACT."""
        for col, cosn, sinn in ((0, "cp", "sp"), (1, "ct", "st"),
                                (3, "cc", "sc")):
            src = apx(dof, col, (L * 4, CHI), (4, L))
            V.add_range_wrap(out=trig[cosn][:], in_=src, shift=0.0,
                             bound=PI, period=2 * PI)
            S.activation(out=trig[sinn][:], in_=trig[cosn][:], func=SIN)
            S.activation(out=aw[:], in_=trig[cosn][:], func=ABS)
            S.activation(out=trig[cosn][:], in_=aw[:], func=SIN,
                         scale=-1.0, bias=halfpi[:])

    def emit_bond(V, stt, trig, X, T, J, F, L):
        """Local 3x3 bond rotations into X slabs (lane layout (chi,j,t))."""
        def ti(nm):
            return apx(trig[nm], 0, (L, CHI), (T, J), (1, T))

        def xo(e):
            return apx(X, e, (J * 9, CHI), (9, J), (F * 9, T))

        tm1, tm2, tu, tv = (trig["tm1"], trig["tm2"], trig["tu"], trig["tv"])

        def tt(tl):
            return apx(tl, 0, (L, CHI), (T, J), (1, T))

        V.tensor_scalar_mul(out=xo(0), in0=ti("ct"), scalar1=-1.0)       # -ct
        stt(out=xo(1), in0=ti("st"), scalar=-1.0, in1=ti("cc"),
            op0=MUL, op1=MUL)                                            # -st*cc
        V.tensor_mul(out=xo(2), in0=ti("st"), in1=ti("sc"))              # st*sc
        V.tensor_mul(out=xo(3), in0=ti("cp"), in1=ti("st"))              # cp*st
        V.tensor_mul(out=tt(tm1), in0=ti("cp"), in1=ti("ct"))            # u
        V.tensor_mul(out=tt(tm2), in0=ti("sp"), in1=ti("ct"))            # v
        V.tensor_mul(out=tt(tu), in0=tt(tm1), in1=ti("cc"))
        V.tensor_mul(out=tt(tv), in0=ti("sp"), in1=ti("sc"))
        stt(out=xo(4), in0=tt(tu), scalar=-1.0, in1=tt(tv),
            op0=MUL, op1=SUB)                                            # -u*cc-sp*sc
        V.tensor_mul(out=tt(tu), in0=tt(tm1), in1=ti("sc"))
        V.tensor_mul(out=tt(tv), in0=ti("sp"), in1=ti("cc"))
        V.tensor_sub(out=xo(5), in0=tt(tu), in1=tt(tv))                  # u*sc-sp*cc
        V.tensor_mul(out=xo(6), in0=ti("sp"), in1=ti("st"))              # sp*st
        V.tensor_mul(out=tt(tu), in0=tt(tm2), in1=ti("cc"))
        V.tensor_mul(out=tt(tv), in0=ti("cp"), in1=ti("sc"))
        V.tensor_sub(out=xo(7), in0=tt(tv), in1=tt(tu))                  # cp*sc-v*cc
        V.tensor_mul(out=tt(tu), in0=tt(tm2), in1=ti("sc"))
        V.tensor_mul(out=tt(tv), in0=ti("cp"), in1=ti("cc"))
        V.tensor_add(out=xo(8), in0=tt(tu), in1=tt(tv))                  # v*sc+cp*cc

    def emit_scan(V, X, tA, tB, T, F):
        """In-place in-block scan of rotation rows 0,1 (state in X slab t,
        elems 0..5; local row2 in elems 6..8 stays)."""
        for t in range(1, T):
            pb = (t - 1) * F * 9
            cb = t * F * 9
            V.tensor_mul(out=apx(tA, 0, (6, F), (3, 2), (1, 3)),
                         in0=apx(X, pb + 0, (9, F), (3, 2), (0, 3)),
                         in1=apx(X, cb + 0, (9, F), (0, 2), (1, 3)))
            V.tensor_mul(out=apx(tB, 0, (6, F), (3, 2), (1, 3)),
                         in0=apx(X, pb + 1, (9, F), (3, 2), (0, 3)),
                         in1=apx(X, cb + 3, (9, F), (0, 2), (1, 3)))
            V.tensor_add(out=apx(tA, 0, (1, 6 * F)),
                         in0=apx(tA, 0, (1, 6 * F)),
                         in1=apx(tB, 0, (1, 6 * F)))
            V.tensor_mul(out=apx(tB, 0, (6, F), (3, 2), (1, 3)),
                         in0=apx(X, pb + 2, (9, F), (3, 2), (0, 3)),
                         in1=apx(X, cb + 6, (9, F), (0, 2), (1, 3)))
            V.tensor_add(out=apx(X, cb, (9, F), (3, 2), (1, 3)),
                         in0=apx(tA, 0, (6, F), (3, 2), (1, 3)),
                         in1=apx(tB, 0, (6, F), (3, 2), (1, 3)))

    def emit_w(V, X, w, dof, tA, tB, T, J, F, L):
        """w[t, f, c] = d * col0(R_inblock) for every atom (R20 via cross
        product; tA keeps all-slab R20 for the bht assembly)."""
        d_ap = apx(dof, 2, (4, T), (L * 4, CHI), (T * 4, J))
        V.tensor_mul(out=apx(tA, 0, (F, T), (1, F)),
                     in0=apx(X, 1, (F * 9, T), (9, F)),
                     in1=apx(X, 5, (F * 9, T), (9, F)))
        V.tensor_mul(out=apx(tB, 0, (F, T), (1, F)),
                     in0=apx(X, 2, (F * 9, T), (9, F)),
                     in1=apx(X, 4, (F * 9, T), (9, F)))
        V.tensor_sub(out=apx(tA, 0, (1, F * T)),
                     in0=apx(tA, 0, (1, F * T)),
                     in1=apx(tB, 0, (1, F * T)))
        V.tensor_mul(out=apx(w, 2, (F * 3, T), (J * 3, CHI), (3, J)),
                     in0=apx(tA, 0, (F, T), (J, CHI), (1, J)),
                     in1=d_ap)
        V.tensor_mul(out=apx(w, 0, (F * 3, T), (J * 3, CHI), (3, J)),
                     in0=apx(X, 0, (F * 9, T), (J * 9, CHI), (9, J)),
                     in1=d_ap)
        V.tensor_mul(out=apx(w, 1, (F * 3, T), (J * 3, CHI), (3, J)),
                     in0=apx(X, 3, (F * 9, T), (J * 9, CHI), (9, J)),
                     in1=d_ap)

    def emit_cumsum(V, w, T, F):
        for t in range(1, T):
            V.tensor_add(out=apx(w, t * F * 3, (1, F * 3)),
                         in0=apx(w, t * F * 3, (1, F * 3)),
                         in1=apx(w, (t - 1) * F * 3, (1, F * 3)))

    def emit_bht(V, X, w, bht, tA, tB, T, F):
        """Assemble 12-elem (3x4 row-major) block-total HTs from the scan
        state at slab T-1 (+ row2 cross products; R20 reused from tA)."""
        base = (T - 1) * F * 9
        V.tensor_copy(out=apx(bht, 0, (12, F), (4, 2), (1, 3)),
                      in_=apx(X, base, (9, F), (3, 2), (1, 3)))
        V.tensor_copy(out=apx(bht, 8, (12, F)),
                      in_=apx(tA, (T - 1) * F, (1, F)))
        # r21 = r02*r10 - r00*r12 ; r22 = r00*r11 - r01*r10
        for dst, (i1, i2), (i3, i4) in ((9, (2, 3), (0, 5)),
                                        (10, (0, 4), (1, 3))):
            V.tensor_mul(out=apx(tA, 0, (1, F)),
                         in0=apx(X, base + i1, (9, F)),
                         in1=apx(X, base + i2, (9, F)))
            V.tensor_mul(out=apx(tB, 0, (1, F)),
                         in0=apx(X, base + i3, (9, F)),
                         in1=apx(X, base + i4, (9, F)))
            V.tensor_sub(out=apx(bht, dst, (12, F)),
                         in0=apx(tA, 0, (1, F)),
                         in1=apx(tB, 0, (1, F)))
        V.tensor_copy(out=apx(bht, 3, (12, F), (4, 3)),
                      in_=apx(w, (T - 1) * F * 3, (3, F), (1, 3)))

    def emit_levels(V, SC, bht, lp2, spx, rx, tA, tB, S, U, seed_rbr=None):
        """level2 (supers), level3 (exclusive over supers), excl_blocks."""
        CS = CHI * S
        LPS = (U + 1) * 12
        V.memset(lp2[:], 0.0)
        V.memset(apx(lp2, 0, (LPS, CS), (5, 3)), 1.0)
        SC.copy(out=apx(lp2, 12, (LPS, CS), (1, 12)),
                in_=apx(bht, 0, (U * 12, CS), (1, 12)))
        for u in range(1, U):
            compose_1d(V, CS,
                       a_off=u * 12, a_step=LPS,
                       b_off=u * 12, b_step=U * 12,
                       o_off=(u + 1) * 12, o_step=LPS,
                       tA=tA, tB=tB, a_tile=lp2, b_tile=bht, o_tile=lp2)
        if seed_rbr is None:
            V.memset(spx[:], 0.0)
            V.memset(apx(spx, 0, (S * 12, CHI), (5, 3)), 1.0)
        else:
            V.tensor_copy(out=apx(spx, 0, (S * 12, CHI), (1, 12)),
                          in_=apx(seed_rbr, 0, (12, CHI), (1, 12)))
        for s in range(1, S):
            compose_1d(V, CHI,
                       a_off=(s - 1) * 12, a_step=S * 12,
                       b_off=(s - 1) * LPS + U * 12, b_step=S * LPS,
                       o_off=s * 12, o_step=S * 12,
                       tA=tA, tB=tB, a_tile=spx, b_tile=lp2, o_tile=spx)
        excl_blocks(V, CS, U, LPS, spx, lp2, rx, tA, tB)

    def emit_down(V, w, rx, xyz, tA, tB, T, F):
        """xyz[f, t, i] = (R_bexcl @ w_cum)[i] + t_bexcl[i]."""
        for i in range(3):
            V.tensor_mul(out=apx(tA, 0, (T, F), (1, T)),
                         in0=apx(rx, 4 * i + 0, (12, F), (0, T)),
                         in1=apx(w, 0, (3, F), (F * 3, T)))
            V.tensor_mul(out=apx(tB, 0, (T, F), (1, T)),
                         in0=apx(rx, 4 * i + 1, (12, F), (0, T)),
                         in1=apx(w, 1, (3, F), (F * 3, T)))
            V.tensor_add(out=apx(tA, 0, (1, F * T)),
                         in0=apx(tA, 0, (1, F * T)),
                         in1=apx(tB, 0, (1, F * T)))
            V.tensor_mul(out=apx(tB, 0, (T, F), (1, T)),
                         in0=apx(rx, 4 * i + 2, (12, F), (0, T)),
                         in1=apx(w, 2, (3, F), (F * 3, T)))
            V.tensor_add(out=apx(tB, 0, (T, F), (1, T)),
                         in0=apx(tB, 0, (T, F), (1, T)),
                         in1=apx(rx, 4 * i + 3, (12, F), (0, T)))
            V.tensor_add(out=apx(xyz, i, (T * 3, F), (3, T)),
                         in0=apx(tA, 0, (T, F), (1, T)),
                         in1=apx(tB, 0, (T, F), (1, T)))

    with tile.TileContext(nc) as tc:
      for _rep in range(repeat):
        with tc.tile_pool(name="main", bufs=1) as mp:
            w0 = mp.tile([P, T0 * F0 * 3], f32)
            d0 = None  # dofs kept resident instead (col 2 read strided)
            tA0 = mp.tile([P, max(T0 * F0, F0 * 12)], f32)
            tB0 = mp.tile([P, max(T0 * F0, F0 * 12)], f32)
            rx0 = mp.tile([P, F0 * 12], f32)
            rbr = mp.tile([P, CHI * 12], f32)
            a32 = mp.tile([P, CHI * 12], f32)
            # jump machinery (tiny)
            jd = mp.tile([P, CHI * 9], f32)
            jang = mp.tile([P, CHI * 2 * 3], f32)
            jsin = mp.tile([P, CHI * 2 * 3], f32)
            jcos = mp.tile([P, CHI * 2 * 3], f32)
            re_ = mp.tile([P, CHI * 2 * 9], f32)
            rj = mp.tile([P, CHI * 9], f32)
            jtmp = mp.tile([P, CHI * 2 * 9], f32)
            halfpi = mp.tile([P, 1], f32)

            nc.sync.dma_start(out=jd[:], in_=jd_d[:])
            nc.vector.memset(halfpi[:], PI / 2)

            V = nc.vector
            SC = nc.scalar
            stt = V.scalar_tensor_tensor

            # ======================= GEN 0 =======================
            with tc.tile_pool(name="pg0", bufs=1) as pg:
                X0 = pg.tile([P, T0 * F0 * 9], f32)
                bht0 = pg.tile([P, F0 * 12], f32)
                lp2_0 = pg.tile([P, CHI * S0 * (U0 + 1) * 12], f32)
                spx0 = pg.tile([P, CHI * S0 * 12], f32)
                dof0 = pg.tile([P, CHI * L0 * 4], f32)

                src = AP(b0_d, 0, [[L0 * 4, P], [P * L0 * 4, CHI], [1, L0 * 4]])
                dst = AP(dof0[:].tensor, dof0[:].offset,
                         [[dof0[:].ap[0][0], P], [L0 * 4, CHI], [1, L0 * 4]])
                nc.sync.dma_start(out=dst, in_=src)

                with tc.tile_pool(name="ptrig0", bufs=1) as pt:
                    trig = {nm: pt.tile([P, CHI * L0], f32, name=f"t0_{nm}")
                            for nm in ("cp", "sp", "ct", "st", "cc", "sc",
                                       "tm1", "tm2", "tu", "tv")}
                    aw = pt.tile([P, CHI * L0], f32)

                    emit_trig(V, SC, dof0, trig, aw, L0, halfpi)
                    emit_bond(V, stt, trig, X0, T0, J0, F0, L0)

                # ---- JUMP HT rotation for chain-start lanes ----
                V.tensor_copy(out=jang[:], in_=apx(jd, 3, (9, CHI), (3, 2),
                                                   (1, 3)))
                V.add_range_wrap(out=jsin[:], in_=jang[:], shift=0.0, bound=PI,
                                 period=2 * PI)
                SC.activation(out=jsin[:], in_=jsin[:], func=SIN)
                V.add_range_wrap(out=jcos[:], in_=jang[:], shift=PI / 2,
                                 bound=PI, period=2 * PI)
                SC.activation(out=jcos[:], in_=jcos[:], func=SIN)

                CR = CHI * 2

                def sc_(tl, ang):
                    return apx(tl, ang, (3, CR))

                def re(e):
                    return apx(re_, e, (9, CR))

                def jt1(e):
                    return apx(jtmp, e, (9, CR))

                sa = lambda: sc_(jsin, 0)
                sb = lambda: sc_(jsin, 1)
                s_c = lambda: sc_(jsin, 2)
                ca = lambda: sc_(jcos, 0)
                cb = lambda: sc_(jcos, 1)
                c_c = lambda: sc_(jcos, 2)
                # R = Rz(c)Ry(b)Rx(a) per (chi, rot) lane
                V.tensor_mul(out=re(0), in0=c_c(), in1=cb())
                V.tensor_mul(out=jt1(0), in0=sb(), in1=sa())
                V.tensor_mul(out=jt1(1), in0=sb(), in1=ca())
                V.tensor_mul(out=jt1(2), in0=c_c(), in1=jt1(0))
                V.tensor_mul(out=jt1(3), in0=s_c(), in1=ca())
                V.tensor_sub(out=re(1), in0=jt1(2), in1=jt1(3))
                V.tensor_mul(out=jt1(2), in0=c_c(), in1=jt1(1))
                V.tensor_mul(out=jt1(3), in0=s_c(), in1=sa())
                V.tensor_add(out=re(2), in0=jt1(2), in1=jt1(3))
                V.tensor_mul(out=re(3), in0=s_c(), in1=cb())
                V.tensor_mul(out=jt1(2), in0=s_c(), in1=jt1(0))
                V.tensor_mul(out=jt1(3), in0=c_c(), in1=ca())
                V.tensor_add(out=re(4), in0=jt1(2), in1=jt1(3))
                V.tensor_mul(out=jt1(2), in0=s_c(), in1=jt1(1))
                V.tensor_mul(out=jt1(3), in0=c_c(), in1=sa())
                V.tensor_sub(out=re(5), in0=jt1(2), in1=jt1(3))
                V.tensor_scalar_mul(out=re(6), in0=sb(), scalar1=-1.0)
                V.tensor_mul(out=re(7), in0=cb(), in1=sa())
                V.tensor_mul(out=re(8), in0=cb(), in1=ca())
                # rj = R1 @ R2 (3x3), lanes = chi
                V.tensor_mul(
                    out=apx(rj, 0, (9, CHI), (3, 3), (1, 3)),
                    in0=apx(re_, 0, (18, CHI), (3, 3), (0, 3)),
                    in1=apx(re_, 9, (18, CHI), (0, 3), (1, 3)))
                V.tensor_mul(
                    out=apx(jtmp, 0, (9, CHI), (3, 3), (1, 3)),
                    in0=apx(re_, 1, (18, CHI), (3, 3), (0, 3)),
                    in1=apx(re_, 12, (18, CHI), (0, 3), (1, 3)))
                V.tensor_add(out=rj[:, : CHI * 9], in0=rj[:, : CHI * 9],
                             in1=jtmp[:, : CHI * 9])
                V.tensor_mul(
                    out=apx(jtmp, 0, (9, CHI), (3, 3), (1, 3)),
                    in0=apx(re_, 2, (18, CHI), (3, 3), (0, 3)),
                    in1=apx(re_, 15, (18, CHI), (0, 3), (1, 3)))
                V.tensor_add(out=rj[:, : CHI * 9], in0=rj[:, : CHI * 9],
                             in1=jtmp[:, : CHI * 9])
                # jump rotation rows 0,1 -> X0 slab 0, lane f=chi*J0 (j=0);
                # full 3x3 so emit_w's cross product sees the jump row2 too.
                V.tensor_copy(out=apx(X0, 0, (J0 * 9, CHI), (1, 9)),
                              in_=apx(rj, 0, (9, CHI), (1, 9)))

                emit_scan(V, X0, tA0, tB0, T0, F0)
                emit_w(V, X0, w0, dof0, tA0, tB0, T0, J0, F0, L0)
                # jump translation overwrites w at (t=0, j=0) lanes
                V.tensor_copy(out=apx(w0, 0, (J0 * 3, CHI), (1, 3)),
                              in_=apx(jd, 0, (9, CHI), (1, 3)))
                emit_cumsum(V, w0, T0, F0)
                emit_bht(V, X0, w0, bht0, tA0, tB0, T0, F0)
                emit_levels(V, SC, bht0, lp2_0, spx0, rx0, tA0, tB0, S0, U0)

                # branch roots: a32 = full HT of (j=32, t=0) in-block state,
                # rbr = rx0[block 32] @ a32
                V.tensor_copy(out=apx(a32, 0, (12, CHI), (4, 2), (1, 3)),
                              in_=apx(X0, 32 * 9, (J0 * 9, CHI), (3, 2), (1, 3)))
                for dst, (i1, i2), (i3, i4) in ((8, (1, 5), (2, 4)),
                                                (9, (2, 3), (0, 5)),
                                                (10, (0, 4), (1, 3))):
                    V.tensor_mul(out=apx(tA0, 0, (1, CHI)),
                                 in0=apx(X0, 32 * 9 + i1, (J0 * 9, CHI)),
                                 in1=apx(X0, 32 * 9 + i2, (J0 * 9, CHI)))
                    V.tensor_mul(out=apx(tB0, 0, (1, CHI)),
                                 in0=apx(X0, 32 * 9 + i3, (J0 * 9, CHI)),
                                 in1=apx(X0, 32 * 9 + i4, (J0 * 9, CHI)))
                    V.tensor_sub(out=apx(a32, dst, (12, CHI)),
                                 in0=apx(tA0, 0, (1, CHI)),
                                 in1=apx(tB0, 0, (1, CHI)))
                V.tensor_copy(out=apx(a32, 3, (12, CHI), (4, 3)),
                              in_=apx(w0, 32 * 3, (J0 * 3, CHI), (1, 3)))
                compose_1d(V, CHI,
                           a_off=32 * 12, a_step=J0 * 12,
                           b_off=0, b_step=12,
                           o_off=0, o_step=12,
                           tA=tA0, tB=tB0,
                           a_tile=rx0, b_tile=a32, o_tile=rbr)

            with tc.tile_pool(name="pxyz0", bufs=1) as po:
                xyz0 = po.tile([P, F0 * T0 * 3], f32)
                emit_down(V, w0, rx0, xyz0, tA0, tB0, T0, F0)
                nc.sync.dma_start(out=kin0_d[:], in_=xyz0[:])

            # ======================= GEN 1 =======================
            with tc.tile_pool(name="pg1", bufs=1) as pg1:
                X1 = pg1.tile([P, T1 * F1 * 9], f32)
                w1 = pg1.tile([P, T1 * F1 * 3], f32)
                bht1 = pg1.tile([P, F1 * 12], f32)
                lp2_1 = pg1.tile([P, CHI * S1 * (U1 + 1) * 12], f32)
                spx1 = pg1.tile([P, CHI * S1 * 12], f32)
                rx1 = pg1.tile([P, F1 * 12], f32)
                dof1 = pg1.tile([P, CHI * L1 * 4], f32)

                src = AP(b1_d, 0, [[L1 * 4, P], [P * L1 * 4, CHI], [1, L1 * 4]])
                dst = AP(dof1[:].tensor, dof1[:].offset,
                         [[dof1[:].ap[0][0], P], [L1 * 4, CHI], [1, L1 * 4]])
                nc.sync.dma_start(out=dst, in_=src)

                with tc.tile_pool(name="ptrig1", bufs=1) as pt1:
                    trig1 = {nm: pt1.tile([P, CHI * L1], f32, name=f"t1_{nm}")
                             for nm in ("cp", "sp", "ct", "st", "cc", "sc",
                                        "tm1", "tm2", "tu", "tv")}
                    aw1 = pt1.tile([P, CHI * L1], f32)

                    emit_trig(V, SC, dof1, trig1, aw1, L1, halfpi)
                    emit_bond(V, stt, trig1, X1, T1, J1, F1, L1)

                emit_scan(V, X1, tA0, tB0, T1, F1)
                emit_w(V, X1, w1, dof1, tA0, tB0, T1, J1, F1, L1)
                emit_cumsum(V, w1, T1, F1)
                emit_bht(V, X1, w1, bht1, tA0, tB0, T1, F1)
                emit_levels(V, SC, bht1, lp2_1, spx1, rx1, tA0, tB0, S1, U1,
                            seed_rbr=rbr)

                xyz1 = pg1.tile([P, F1 * T1 * 3], f32)
                emit_down(V, w1, rx1, xyz1, tA0, tB0, T1, F1)
                nc.sync.dma_start(out=kin1_d[:], in_=xyz1[:])

    nc.compile()
    return nc


def get_program(repeat=1):
    key = ("nc", repeat)
    if key not in _CACHE:
        _CACHE[key] = _build_program(repeat)
    return _CACHE[key]


# ------------------------------------------------------------------- host
def _shard_inputs(dofs, doftype):
    """Build the 8 per-core input maps (lane order (p, chi, j, t))."""
    in_maps = []
    chain_starts = 1 + np.arange(C0, dtype=np.int64) * L0
    jd_all = np.ascontiguousarray(dofs[chain_starts])       # [C0, 9]
    for core in range(NCORES):
        g0 = np.ascontiguousarray(
            dofs[1 + core * A0: 1 + (core + 1) * A0, :4])
        g1 = np.ascontiguousarray(
            dofs[BOFF + core * A1: BOFF + (core + 1) * A1, :4])
        jd = np.ascontiguousarray(
            jd_all[core * CH0:(core + 1) * CH0]
            .reshape(CHI, P, 9).transpose(1, 0, 2).reshape(P, CHI * 9))
        in_maps.append({"b0": g0, "b1": g1, "jd": jd})
    return in_maps


def _lane_ids(id_idx, core):
    """id_idx values of this core's atoms in device lane order (p, f, t)."""
    ids0 = (id_idx[core * A0:(core + 1) * A0]
            .reshape(CHI, P, L0).transpose(1, 0, 2).ravel())
    ids1 = (id_idx[BOFF - 1 + core * A1: BOFF - 1 + (core + 1) * A1]
            .reshape(CHI, P, L1).transpose(1, 0, 2).ravel())
    return ids0, ids1


def _structure_ok(doftype, gen0_paths, gen1_paths):
    chain_starts = 1 + np.arange(C0, dtype=np.int64) * L0
    g0 = np.concatenate(
        [np.zeros((C0, 1), np.int64), chain_starts[:, None] + np.arange(L0)],
        axis=1)
    if not np.array_equal(gen0_paths, g0.astype(gen0_paths.dtype)):
        return False
    branch_roots = chain_starts + L0 // 2
    g1 = np.concatenate(
        [branch_roots[:, None],
         BOFF + (np.arange(C1, dtype=np.int64) * L1)[:, None] + np.arange(L1)],
        axis=1)
    if not np.array_equal(gen1_paths, g1.astype(gen1_paths.dtype)):
        return False
    if doftype[0] != 0:
        return False
    if not np.all(doftype[chain_starts] == 1):
        return False
    dt = doftype.copy()
    dt[chain_starts] = 2
    if not np.all(dt[1:] == 2):
        return False
    return True


def _numpy_fallback(dofs, doftype, gen0_paths, gen1_paths, id_idx):
    """Exact numpy port of the reference (slow path, safety net)."""
    def rx(a):
        c, s = np.cos(a), np.sin(a)
        o, z = np.ones_like(a), np.zeros_like(a)
        return np.stack([np.stack([o, z, z, z], -1), np.stack([z, c, -s, z], -1),
                         np.stack([z, s, c, z], -1), np.stack([z, z, z, o], -1)], -2)

    def ry(a):
        c, s = np.cos(a), np.sin(a)
        o, z = np.ones_like(a), np.zeros_like(a)
        return np.stack([np.stack([c, z, s, z], -1), np.stack([z, o, z, z], -1),
                         np.stack([-s, z, c, z], -1), np.stack([z, z, z, o], -1)], -2)

    def rz(a):
        c, s = np.cos(a), np.sin(a)
        o, z = np.ones_like(a), np.zeros_like(a)
        return np.stack([np.stack([c, -s, z, z], -1), np.stack([s, c, z, z], -1),
                         np.stack([z, z, o, z], -1), np.stack([z, z, z, o], -1)], -2)

    def trans(x, y, z):
        o, zr = np.ones_like(x), np.zeros_like(x)
        return np.stack([np.stack([o, zr, zr, x], -1), np.stack([zr, o, zr, y], -1),
                         np.stack([zr, zr, o, z], -1), np.stack([zr, zr, zr, o], -1)], -2)

    dofs = dofs.astype(np.float32)
    phi_p, theta, d, phi_c = dofs[:, 0], dofs[:, 1], dofs[:, 2], dofs[:, 3]
    z = np.zeros_like(d)
    bond = rx(phi_p) @ rz(np.pi - theta) @ trans(d, z, z) @ rx(phi_c)
    rot = lambda a, b, c: rz(c) @ ry(b) @ rx(a)
    jump = (trans(dofs[:, 0], dofs[:, 1], dofs[:, 2])
            @ rot(dofs[:, 3], dofs[:, 4], dofs[:, 5])
            @ rot(dofs[:, 6], dofs[:, 7], dofs[:, 8]))
    eye = np.broadcast_to(np.eye(4, dtype=dofs.dtype), bond.shape)
    dt = doftype[:, None, None]
    hts = np.where(dt == 1, jump, np.where(dt == 2, bond, eye)).astype(np.float32)
    for paths in (gen0_paths, gen1_paths):
        seg = hts[paths]
        out = np.empty_like(seg)
        out[:, 0] = seg[:, 0]
        for i in range(1, seg.shape[1]):
            out[:, i] = out[:, i - 1] @ seg[:, i]
        hts[paths] = out
    kincoords = hts[:, :3, 3]
    coords = np.zeros((N - 1, 3), dtype=dofs.dtype)
    coords[np.asarray(id_idx)] = kincoords[1:]
    return coords


def kernel(dofs, doftype, gen0_paths, gen1_paths, id_idx):
    dofs = np.asarray(dofs, dtype=np.float32)
    doftype = np.asarray(doftype, dtype=np.int32)
    gen0_paths = np.asarray(gen0_paths)
    gen1_paths = np.asarray(gen1_paths)
    id_idx = np.asarray(id_idx, dtype=np.int32)

    if not _structure_ok(doftype, gen0_paths, gen1_paths):
        return _numpy_fallback(dofs, doftype, gen0_paths, gen1_paths, id_idx)

    from concourse.bass_utils import run_bass_kernel_spmd

    nc = get_program()
    in_maps = _shard_inputs(dofs, doftype)
    res = run_bass_kernel_spmd(nc, in_maps, core_ids=list(range(NCORES)))
    out = np.empty((N - 1, 3), dtype=np.float32)
    for core in range(NCORES):
        ids0, ids1 = _lane_ids(id_idx, core)
        out[ids0] = res.results[core]["kin0"].reshape(-1, 3)
        out[ids1] = res.results[core]["kin1"].reshape(-1, 3)
    return out
